# revision 3
# baseline (speedup 1.0000x reference)
"""Batch triplet loss on 8 TRN2 NeuronCores — fp8 DoubleRow half-Gram, v9.

v8 -> v9: the fold matmul (-0.5*sq_j into PSUM, 72 N=512 matmuls) is
replaced by a gpsimd SBUF->SBUF broadcast DMA producing rep[p, j] = sq_j
(f16) once per slab, plus a DVE scalar_tensor_tensor consumer:
  ft = (psd * -2) + rep = sq_j - 2*dot         (row-max -> m1, host adds sq_i)
  d2 slabs also: ft2 = ACT(-2*psd + sq_i bias) (facc max; partition-max -> m2,
                                                host adds sq_j)
This removes all fold matmuls + their LDWEIGHTS from the PE stream; the PE
now runs only the 288 DoubleRow Gram matmuls.
"""

import os
from contextlib import ExitStack

import ml_dtypes
import numpy as np

import concourse.bass as bass
import concourse.tile as tile
from concourse import bacc, bass_isa, bass_utils, mybir

N = 8192
D = 1024
NCORES = 8
OWN = N // NCORES       # 1024
KT = D // 128           # 8
JW = 512
NPAN = 5
MOVW = NPAN * OWN       # 5120
NSLAB = MOVW // JW      # 10
NCOL = 9                # m1cols columns per it (slabs 8/9 share col 8)
IT = OWN // 128         # 8
EPS = 1e-6
MARGIN = 0.5

F8 = mybir.dt.float8e4
F16 = mybir.dt.float16
F32 = mybir.dt.float32

_NC = None

D2S = set(range(2, 10))
S_CHUNKS = [[0, 1, 2, 3, 4], [5, 6, 7, 8, 9]]


def _slab_its(s):
    if s == 8:
        return range(0, IT // 2)
    if s == 9:
        return range(IT // 2, IT)
    return range(IT)


def _build_nc():
    REPEAT = int(os.environ.get("KBENCH_REPEAT", "1"))
    HWLOOP = int(os.environ.get("KBENCH_HWLOOP", "0"))  # hw-loop pair count
    nc = bacc.Bacc("TRN2", target_bir_lowering=False, debug=False)
    mov = nc.dram_tensor("mov", [128, KT * MOVW], F8, kind="ExternalInput").ap()
    sqd = nc.dram_tensor("sqd", [1, MOVW], F32, kind="ExternalInput").ap()
    sqid = nc.dram_tensor("sqid", [128, IT], F32, kind="ExternalInput").ap()
    out_m1 = nc.dram_tensor("out_m1", [128, IT], F32, kind="ExternalOutput").ap()
    out_m2 = nc.dram_tensor("out_m2", [1, 8 * JW], F16, kind="ExternalOutput").ap()

    mov_v = mov.rearrange("p (k w) -> p k w", k=KT)  # [128, KT, MOVW]

    with ExitStack() as ctx:
        tc = ctx.enter_context(tile.TileContext(nc))
        big = ctx.enter_context(tc.tile_pool(name="big", bufs=1))
        ftp = ctx.enter_context(tc.tile_pool(name="ftp", bufs=4))
        ft2p = ctx.enter_context(tc.tile_pool(name="ft2p", bufs=4))
        facp = ctx.enter_context(tc.tile_pool(name="facp", bufs=8))
        parp = ctx.enter_context(tc.tile_pool(name="parp", bufs=2))
        resp = ctx.enter_context(tc.tile_pool(name="resp", bufs=1))
        ps_mm = ctx.enter_context(tc.tile_pool(name="ps_mm", bufs=8, space="PSUM"))

        movs2 = [
            big.tile([128, KT * MOVW], F8, tag=f"mv{h}", name=f"movs_all{h}")
            for h in range(2)
        ]
        rep2 = [
            big.tile([128, MOVW], F16, tag=f"rep{h}", name=f"rep{h}") for h in range(2)
        ]
        sqs = resp.tile([1, MOVW], F32, tag="sqs", name="sqs")
        sqh = resp.tile([1, MOVW], F16, tag="sqh", name="sqh")
        sqi2 = [
            resp.tile([128, IT], F32, tag=f"sqi{h}", name=f"sqi{h}") for h in range(2)
        ]
        m1cols = resp.tile([128, IT * NCOL], F16, tag="m1cols", name="m1cols")
        msb = resp.tile([128, IT], F32, tag="msb", name="msb")
        m2sb = resp.tile([1, 8 * JW], F16, tag="m2sb", name="m2sb")

        def emit_rep(rep_i):
            mv = movs2[rep_i % 2][:].rearrange("p (k w) -> p k w", k=KT)
            rep = rep2[rep_i % 2]
            sqi = sqi2[rep_i % 2]
            nc.sync.dma_start(sqs[:], sqd[:])
            nc.sync.dma_start(sqi[:], sqid[:])
            for n in range(NPAN):
                nc.sync.dma_start(
                    mv[:, :, n * OWN : (n + 1) * OWN],
                    mov_v[:, :, n * OWN : (n + 1) * OWN],
                )
            nc.scalar.copy(sqh[:], sqs[:])
            for s in range(NSLAB):
                nc.gpsimd.partition_broadcast(
                    rep[:, s * JW : (s + 1) * JW],
                    sqh[:, s * JW : (s + 1) * JW],
                    channels=128,
                )

            facc = {}
            for s in sorted(D2S):
                f = facp.tile([128, JW], F16, tag="facc", name=f"facc{rep_i}_{s}")
                nc.vector.memset(f[:], -60000.0)
                facc[s] = f

            for it in range(IT):
                for chunk in S_CHUNKS:
                    active = [s for s in chunk if it in _slab_its(s)]
                    psds = {}
                    for s in active:
                        psds[s] = ps_mm.tile(
                            [128, JW], F32, tag="psd", name=f"psd{rep_i}_{it}_{s}"
                        )
                    for t in range(KT // 2):
                        for s in active:
                            nc.tensor.matmul(
                                psds[s][:],
                                mv[:, 2 * t : 2 * t + 2, it * 128 : (it + 1) * 128],
                                mv[:, 2 * t : 2 * t + 2, s * JW : (s + 1) * JW],
                                start=(t == 0),
                                stop=(t == KT // 2 - 1),
                                perf_mode=mybir.MatmulPerfMode.DoubleRow,
                            )
                    for s in active:
                        ft = ftp.tile([128, JW], F16, tag="ft", name=f"ft{rep_i}_{it}_{s}")
                        nc.vector.scalar_tensor_tensor(
                            out=ft[:],
                            in0=psds[s][:],
                            scalar=-2.0,
                            in1=rep[:, s * JW : (s + 1) * JW],
                            op0=mybir.AluOpType.mult,
                            op1=mybir.AluOpType.add,
                        )
                        col = it * NCOL + min(s, 8)
                        nc.vector.reduce_max(
                            m1cols[:, col : col + 1],
                            ft[:],
                            axis=mybir.AxisListType.X,
                            op=mybir.AluOpType.max,
                        )
                        if s in facc:
                            ft2 = ft2p.tile(
                                [128, JW], F16, tag="ft2", name=f"ft2_{rep_i}_{it}_{s}"
                            )
                            nc.scalar.activation(
                                ft2[:],
                                psds[s][:],
                                mybir.ActivationFunctionType.Identity,
                                bias=sqi[:, it : it + 1],
                                scale=-2.0,
                            )
                            nc.vector.tensor_tensor(
                                facc[s][:], facc[s][:], ft2[:], op=mybir.AluOpType.max
                            )

            for s in sorted(D2S):
                par = parp.tile([128, JW], F16, tag="par", name=f"par{rep_i}_{s}")
                nc.gpsimd.partition_all_reduce(
                    par[:], facc[s][:], channels=128, reduce_op=bass_isa.ReduceOp.max
                )
                nc.scalar.copy(m2sb[:, (s - 2) * JW : (s - 1) * JW], par[0:1, :])

            for it in range(IT):
                nc.vector.reduce_max(
                    msb[:, it : it + 1],
                    m1cols[:, it * NCOL : (it + 1) * NCOL],
                    axis=mybir.AxisListType.X,
                    op=mybir.AluOpType.max,
                )

        if HWLOOP:
            with tc.For_i(0, HWLOOP, 1):
                emit_rep(0)
                emit_rep(1)
        else:
            for rep_i in range(REPEAT):
                emit_rep(rep_i)

        nc.gpsimd.dma_start(out_m1[:], msb[:])
        nc.gpsimd.dma_start(out_m2[:], m2sb[:])

    nc.compile()
    return nc


def _get_nc():
    global _NC
    if _NC is None:
        _NC = _build_nc()
    return _NC


def _core_cols(c):
    """Global column indices (batch rows) for core c's MOVW columns."""
    segs = [np.arange(OWN) + ((c + d) % NCORES) * OWN for d in range(4)]
    p4 = ((c + 4) % NCORES) * OWN
    if c < 4:
        segs += [p4 + np.arange(512), p4 + 512 + np.arange(512)]
    else:
        segs += [p4 + 512 + np.arange(512), p4 + np.arange(512)]
    return np.concatenate(segs)


def _make_in_maps(batch, positive):
    bT = np.ascontiguousarray(batch.T)  # [D, N] f32
    b8 = bT.astype(ml_dtypes.float8_e4m3)
    sq_full = np.einsum("ij,ij->j", bT, bT, dtype=np.float32)  # [N] exact
    g = np.ascontiguousarray(b8.reshape(KT, 128, N).transpose(1, 0, 2))
    in_maps = []
    for c in range(NCORES):
        cols = _core_cols(c)
        mov_c = np.ascontiguousarray(g[:, :, cols]).reshape(128, KT * MOVW)
        sqd_c = np.ascontiguousarray(sq_full[cols].reshape(1, MOVW))
        sqi_c = np.ascontiguousarray(
            sq_full[c * OWN : (c + 1) * OWN].reshape(IT, 128).T
        )
        in_maps.append({"mov": mov_c, "sqd": sqd_c, "sqid": sqi_c})
    return in_maps, sq_full


def _combine(results, batch, positive, sq_full):
    f32 = np.float32
    # m1 rows hold max_j(sq_j - 2 dot); add own sq_i
    d2max = np.concatenate(
        [results[c]["out_m1"].T.reshape(-1) for c in range(NCORES)]
    ).astype(f32) + sq_full
    # m2 segs hold max_i(sq_i - 2 dot) for target row j; add sq_j
    for s in range(2, 10):
        for c in range(NCORES):
            seg = results[c]["out_m2"][0, (s - 2) * JW : (s - 1) * JW].astype(f32)
            if s < 8:
                d_, r_ = s // 2, s % 2
                t = (c + d_) % NCORES
                lo = t * OWN + r_ * JW
            else:
                t = (c + 4) % NCORES
                if s == 8:
                    h = 0 if c < 4 else 1
                else:
                    h = 1 if c < 4 else 0
                lo = t * OWN + h * JW
            cand = seg + sq_full[lo : lo + JW]
            d2max[lo : lo + JW] = np.maximum(d2max[lo : lo + JW], cand)

    deps2 = f32(D * EPS * EPS)
    pp = f32(np.sum(positive.astype(f32) ** 2, dtype=f32))
    dotp = batch @ positive  # [N] exact f32
    max_neg = np.sqrt(np.maximum(d2max + deps2, f32(0.0)))
    pos2 = sq_full - f32(2.0) * dotp + pp
    pos_dist = np.sqrt(np.maximum(pos2 + deps2, f32(0.0)))
    losses = np.maximum(pos_dist - max_neg + f32(MARGIN), f32(0.0))
    valid = ~np.all(batch == positive[None, :], axis=1)
    cnt = f32(valid.sum())
    total = f32(np.sum(losses[valid], dtype=f32))
    return np.asarray(total / cnt, dtype=np.float32)


def run_on_cores(batch, positive, **kwargs):
    nc = _get_nc()
    in_maps, sq_full = _make_in_maps(batch, positive)
    res = bass_utils.run_bass_kernel_spmd(
        nc, in_maps, core_ids=list(range(NCORES)), **kwargs
    )
    return res, sq_full


def kernel(batch, positive):
    batch = np.asarray(batch, dtype=np.float32)
    positive = np.asarray(positive, dtype=np.float32)
    res, sq_full = run_on_cores(batch, positive)
    return _combine(res.results, batch, positive, sq_full)
